# revision 4
# baseline (speedup 1.0000x reference)
"""LSTM Trainium2 kernel: tensor-parallel over hidden across 8 NeuronCores,
h.T all-gather via firmware collective (AllGather, cc_dim=Free) each step.

Same compute structure as kernel_tp (per-core gate slice [64,512], 12 fp32r
matmuls with weights moving, single PE transpose of the own h.T chunk), but
the per-step exchange is: PE transpose -> (ACT-issued HWDGE) DMA straight
from PSUM -> cc_in DRAM -> AllGather (cc_dim=Partition, out [1024,64] in
rank order) -> cc_out (Shared) -> strided DMA -> hT SBUF slots. The 512
collectives are unrolled straight-line on gpsimd (collectives cannot live
inside hardware loops). Arrival counting is round-parity-split (s_hT0/1):
round j+2 cannot be produced before the waiter of round j releases, so
same-parity masking is impossible.
"""
import sys

sys.path.insert(0, "/opt/trn_rl_repo")
import numpy as np

import concourse.bass as bass
import concourse.mybir as mybir

B, I, H = 64, 512, 1024
NC = 8
HC = H // NC
G = 4 * HC  # 512
KT = 12
XB = 4
F32 = mybir.dt.float32
F32R = mybir.dt.float32r
GATE_ORDER = [0, 1, 3, 2]  # [i | f | o | g]


def _wge(eng, rw, var, mul, add, sem):
    eng.reg_mul(rw, var, mul)
    if add:
        eng.reg_add(rw, rw, add)
    eng.wait_ge(sem, rw)


def build(S=512):
    NI = S // 2
    assert NI % XB == 0
    nc = bass.Bass(target_bir_lowering=False, num_devices=NC)

    xT = nc.dram_tensor("xT", [I, S * B], F32R, kind="ExternalInput")
    wcat = nc.dram_tensor("wcat", [H + I, G], F32R, kind="ExternalInput")
    ident = nc.dram_tensor("ident", [B, B], F32, kind="ExternalInput")
    out_hc = nc.dram_tensor("out_hc", [B, 2 * HC], F32, kind="ExternalOutput")
    cc_in = nc.dram_tensor("cc_in", [2 * 128, B], F32R, kind="Internal")
    cc_out = nc.dram_tensor(
        "cc_out", [2 * NC * 128, B], F32R, kind="Internal", addr_space="Shared"
    )

    from contextlib import ExitStack

    es = ExitStack()
    w_sb = es.enter_context(nc.sbuf_tensor("w_sb", [128, KT * G], F32R))
    xb = es.enter_context(nc.sbuf_tensor("xb", [128, XB * 512], F32R))
    hT = es.enter_context(nc.sbuf_tensor("hT", [128, 2 * NC * B], F32R))
    stage = es.enter_context(nc.sbuf_tensor("stage", [128, 2 * B], F32R))
    act = es.enter_context(nc.sbuf_tensor("act", [B, 2 * G], F32))
    c_sb = es.enter_context(nc.sbuf_tensor("c_sb", [B, HC], F32))
    tc_sb = es.enter_context(nc.sbuf_tensor("tc_sb", [B, 2 * HC], F32))
    h_sb = es.enter_context(nc.sbuf_tensor("h_sb", [B, HC], F32))
    ig_sb = es.enter_context(nc.sbuf_tensor("ig_sb", [B, HC], F32))
    fc_sb = es.enter_context(nc.sbuf_tensor("fc_sb", [B, HC], F32))
    id_sb = es.enter_context(nc.sbuf_tensor("id_sb", [B, B], F32))
    pga0 = es.enter_context(nc.psum_tensor("pga0", [B, 256], F32))
    pga1 = es.enter_context(nc.psum_tensor("pga1", [B, 256], F32))
    pgb0 = es.enter_context(nc.psum_tensor("pgb0", [B, 256], F32))
    pgb1 = es.enter_context(nc.psum_tensor("pgb1", [B, 256], F32))
    pdum = es.enter_context(nc.psum_tensor("pdum", [B, G], F32))
    pt0 = es.enter_context(nc.psum_tensor("pt0", [128, B], F32))
    pt1 = es.enter_context(nc.psum_tensor("pt1", [128, B], F32))

    s_load = es.enter_context(nc.semaphore("s_load"))
    s_init = es.enter_context(nc.semaphore("s_init"))
    s_x = [es.enter_context(nc.semaphore(f"s_x{m}")) for m in range(XB)]
    s_mm = es.enter_context(nc.semaphore("s_mm"))
    s_act = es.enter_context(nc.semaphore("s_act"))  # primed +2
    s_dc = es.enter_context(nc.semaphore("s_dc"))
    s_tc = es.enter_context(nc.semaphore("s_tc"))
    s_h = es.enter_context(nc.semaphore("s_h"))  # primed +1
    s_tr = es.enter_context(nc.semaphore("s_tr"))
    s_ev = es.enter_context(nc.semaphore("s_ev"))  # primed +1
    # exchange sems
    s_ci = [es.enter_context(nc.semaphore(f"s_ci{q}")) for q in range(2)]
    s_hT = [es.enter_context(nc.semaphore(f"s_hT{q}")) for q in range(2)]
    s_cc = es.enter_context(nc.semaphore("s_cc"))
    s_out = es.enter_context(nc.semaphore("s_out"))

    nc.all_core_barrier()

    with es:
        with nc.Block() as block:
            pga = [pga0, pga1]
            pgb = [pgb0, pgb1]
            pt = [pt0, pt1]

            # Counts (t step, j round in -1..S-2, m = per-parity round index):
            #   s_mm t+1; s_act t+3 (prime2); s_dc,s_tc t+1; s_h t+2 (prime1)
            #   s_tr j+2; s_cc j+2
            #   s_ci[q] 16*(m+1) after in-DMA of m-th parity-q round
            #   s_hT[q] 16*(m+1) after out-DMA of m-th parity-q round
            # parity q of round j: j&1 (round -1 -> q=1, m=0; j=2i -> q=0, m=i;
            # j=2i+1 -> q=1, m=i+1)

            @block.sync
            def _(sync):
                with (
                    sync.register("rxa") as rxa,
                    sync.register("rxb") as rxb,
                    sync.register("rxc") as rxc,
                    sync.register("rxd") as rxd,
                    sync.register("rw") as rw,
                ):
                    sync.nop().then_inc(s_act, 2)
                    sync.nop().then_inc(s_h, 1)
                    sync.nop().then_inc(s_ev, 1)
                    for k in range(KT):
                        sync.dma_start(
                            w_sb[:, k * G : (k + 1) * G],
                            wcat[k * 128 : (k + 1) * 128, :],
                        ).then_inc(s_load, 16)
                    sync.dma_start(id_sb[:, :], ident[:, :]).then_inc(s_load, 16)
                    for m in range(XB):
                        for xk in range(4):
                            sync.dma_start(
                                xb[:, m * 512 + xk * 128 : m * 512 + (xk + 1) * 128],
                                xT[xk * 128 : (xk + 1) * 128, m * 128 : (m + 1) * 128],
                            ).then_inc(s_x[m], 16)
                    rx = [rxa, rxb, rxc, rxd]
                    for xk in range(4):
                        sync.reg_mov(rx[xk], xk * 128 * (S * B) + XB * 128)
                    if NI > XB:
                        with sync.Fori(0, (NI - XB) // XB) as o:
                            for u in range(XB):
                                _wge(sync, rw, o, 4 * XB, 4 * u + 4, s_mm)
                                for xk in range(4):
                                    sync.dma_start(
                                        bass.AP(
                                            xb,
                                            u * 512 + xk * 128,
                                            [[XB * 512, 128], [1, 128]],
                                        ),
                                        bass.AP(xT, rx[xk], [[S * B, 128], [1, 128]]),
                                    ).then_inc(s_x[u], 16)
                                    sync.reg_add(rx[xk], rx[xk], 128)
                    sync.wait_ge(s_h, S + 1)
                    sync.wait_ge(s_dc, S)
                    sync.dma_start(out_hc[:, 0:HC], h_sb[:, :]).then_inc(s_out, 16)
                    sync.dma_start(out_hc[:, HC : 2 * HC], c_sb[:, :]).then_inc(
                        s_out, 16
                    )
                    sync.wait_ge(s_out, 32)

            @block.tensor
            def _(te):
                with te.register("rw") as rw:
                    te.wait_ge(s_load, 16 * (KT + 1))
                    te.wait_ge(s_init, 2)
                    with te.Fori(0, NI // XB) as o:
                        for u in range(XB):
                            _wge(te, rw, o, 64, 64, s_x[u])
                            for s2 in range(2):
                                p = s2
                                rp = 1 - s2
                                tof = 2 * u + s2
                                _wge(te, rw, o, 2 * XB, tof + 1, s_act)
                                for hf in range(2):
                                    c0 = hf * 256
                                    bank = (pga if hf == 0 else pgb)[p]
                                    for xk in range(4):
                                        cb = u * 512 + xk * 128 + s2 * 64
                                        te.matmul(
                                            bank[:, :],
                                            xb[:, cb : cb + 64],
                                            w_sb[
                                                :,
                                                (8 + xk) * G
                                                + c0 : (8 + xk) * G
                                                + c0
                                                + 256,
                                            ],
                                            start=(xk == 0),
                                            stop=False,
                                        )
                                # pt[rp] free: evac(t-3) done: s_ev >= t
                                _wge(te, rw, o, 2 * XB, tof, s_ev)
                                _wge(te, rw, o, 2 * XB, tof + 1, s_h)
                                te.transpose(
                                    pt[rp][:, :], h_sb[:, :], id_sb[:, :]
                                ).then_inc(s_tr)
                                for dk in range(60):
                                    te.matmul(
                                        pdum[:, :],
                                        xb[:, u * 512 : u * 512 + 64],
                                        w_sb[:, 0:G],
                                        start=True,
                                        stop=True,
                                    )
                                # round t-1 (parity rp) gathered into hT:
                                # s_hT[rp] >= 16*(i+1)
                                _wge(te, rw, o, 16 * XB, 16 * (u + 1), s_hT[rp])
                                for hf in range(2):
                                    c0 = hf * 256
                                    bank = (pga if hf == 0 else pgb)[p]
                                    mm = None
                                    for k in range(8):
                                        hb = rp * NC * B + k * B
                                        mm = te.matmul(
                                            bank[:, :],
                                            hT[:, hb : hb + B],
                                            w_sb[:, k * G + c0 : k * G + c0 + 256],
                                            start=False,
                                            stop=(k == 7),
                                        )
                                    mm.then_inc(s_mm)

            @block.scalar
            def _(sc):
                with sc.register("rw") as rw:
                    Sig = mybir.ActivationFunctionType.Sigmoid
                    Tanh = mybir.ActivationFunctionType.Tanh
                    # peeled round -1: zero h.T chunk -> cc_in parity 1,
                    # collective, gather back into hT parity 1
                    sc.wait_ge(s_ev, 2)
                    sc.dma_start(cc_in[128:256, :], stage[:, B : 2 * B]).then_inc(
                        s_ci[1], 16
                    )
                    sc.wait_ge(s_cc, 1)
                    sc.dma_start(
                        hT[:, NC * B : 2 * NC * B],
                        bass.AP(
                            cc_out,
                            NC * 128 * B,
                            [[B, 128], [128 * B, NC], [1, B]],
                        ),
                    ).then_inc(s_hT[1], 16)
                    with sc.Fori(0, NI - 1) as i:
                        for s2 in range(2):
                            p = s2
                            # t = 2i+s2; round j = t-1 handled after acts
                            _wge(sc, rw, i, 4, 2 * s2 + 1, s_mm)
                            _wge(sc, rw, i, 2, s2, s_h)
                            sc.activation(
                                act[:, p * G : p * G + 256], pga[p][:, :], Sig
                            )
                            _wge(sc, rw, i, 4, 2 * s2 + 2, s_mm)
                            sc.activation(
                                act[:, p * G + 256 : p * G + 384],
                                pgb[p][:, 0:128],
                                Sig,
                            )
                            sc.activation(
                                act[:, p * G + 384 : p * G + 512],
                                pgb[p][:, 128:256],
                                Tanh,
                            ).then_inc(s_act)
                            _wge(sc, rw, i, 2, s2 + 1, s_dc)
                            sc.activation(
                                tc_sb[:, p * HC : (p + 1) * HC], c_sb[:, :], Tanh
                            ).then_inc(s_tc)
                            # ---- exchange DMAs for round j = t (parity p) ----
                            # in-DMA: stage[p] -> cc_in[p]; needs evac(t)
                            # done (s_ev >= t+3) and CC of round t-2 done
                            # reading cc_in[p] (s_cc >= t)
                            _wge(sc, rw, i, 2, s2 + 3, s_ev)
                            _wge(sc, rw, i, 2, s2, s_cc)
                            sc.dma_start(
                                cc_in[p * 128 : (p + 1) * 128, :],
                                stage[:, p * B : (p + 1) * B],
                            ).then_inc(s_ci[p], 16)
                            # out-DMA: cc_out[p] [1024,64] -> hT parity-p
                            # slots [128, 512]; cc(t) done
                            _wge(sc, rw, i, 2, s2 + 2, s_cc)
                            sc.dma_start(
                                hT[:, p * NC * B : (p + 1) * NC * B],
                                bass.AP(
                                    cc_out,
                                    p * NC * 128 * B,
                                    [[B, 128], [128 * B, NC], [1, B]],
                                ),
                            ).then_inc(s_hT[p], 16)
                    # peeled final iteration i = NI-1 (t = S-2, S-1): the
                    # round S-1 exchange does not exist, so only t = S-2
                    # does exchange DMAs
                    Sg, Th = Sig, Tanh
                    for s2 in range(2):
                        p = s2
                        t = S - 2 + s2
                        sc.wait_ge(s_mm, 2 * t + 1)
                        sc.wait_ge(s_h, t)
                        sc.activation(act[:, p * G : p * G + 256], pga[p][:, :], Sg)
                        sc.wait_ge(s_mm, 2 * t + 2)
                        sc.activation(
                            act[:, p * G + 256 : p * G + 384], pgb[p][:, 0:128], Sg
                        )
                        sc.activation(
                            act[:, p * G + 384 : p * G + 512], pgb[p][:, 128:256], Th
                        ).then_inc(s_act)
                        sc.wait_ge(s_dc, t + 1)
                        sc.activation(
                            tc_sb[:, p * HC : (p + 1) * HC], c_sb[:, :], Th
                        ).then_inc(s_tc)
                        if s2 == 0:
                            sc.wait_ge(s_ev, t + 3)
                            sc.wait_ge(s_cc, t)
                            sc.dma_start(
                                cc_in[p * 128 : (p + 1) * 128, :],
                                stage[:, p * B : (p + 1) * B],
                            ).then_inc(s_ci[p], 16)
                            sc.wait_ge(s_cc, t + 2)
                            sc.dma_start(
                                hT[:, p * NC * B : (p + 1) * NC * B],
                                bass.AP(
                                    cc_out,
                                    p * NC * 128 * B,
                                    [[B, 128], [128 * B, NC], [1, B]],
                                ),
                            ).then_inc(s_hT[p], 16)

            @block.vector
            def _(vec):
                mult = mybir.AluOpType.mult
                add = mybir.AluOpType.add
                with vec.register("rw") as rw:
                    vec.memset(h_sb[:, :], 0).then_inc(s_init)
                    vec.memset(c_sb[:, :], 0).then_inc(s_init)
                    # peeled evac(-1): zero h.T -> stage parity 1
                    vec.wait_ge(s_tr, 1)
                    vec.tensor_copy(stage[:, B : 2 * B], pt1[:, :]).then_inc(s_ev)
                    with vec.Fori(0, NI - 1) as i:
                        for s2 in range(2):
                            p = s2
                            _wge(vec, rw, i, 2, s2 + 3, s_act)
                            vec.tensor_tensor(
                                ig_sb[:, :],
                                act[:, p * G : p * G + 128],
                                act[:, p * G + 384 : p * G + 512],
                                mult,
                            )
                            vec.tensor_tensor(
                                fc_sb[:, :],
                                act[:, p * G + 128 : p * G + 256],
                                c_sb[:, :],
                                mult,
                            )
                            vec.tensor_tensor(
                                c_sb[:, :], ig_sb[:, :], fc_sb[:, :], add
                            ).then_inc(s_dc)
                            _wge(vec, rw, i, 2, s2 + 1, s_tc)
                            _wge(vec, rw, i, 2, s2 + 1, s_tr)
                            vec.tensor_tensor(
                                h_sb[:, :],
                                act[:, p * G + 256 : p * G + 384],
                                tc_sb[:, p * HC : (p + 1) * HC],
                                mult,
                            ).then_inc(s_h)
                            # evac(t): transpose(t) done; stage[p] free when
                            # in-DMA of round t-2 drained
                            _wge(vec, rw, i, 2, s2 + 2, s_tr)
                            _wge(vec, rw, i, 16, 16 * s2, s_ci[p])
                            vec.tensor_copy(
                                stage[:, p * B : (p + 1) * B], pt[p][:, :]
                            ).then_inc(s_ev)
                    # peeled final iteration (t = S-2, S-1): no evac for the
                    # last step (round S-1 never exchanged)
                    for s2 in range(2):
                        p = s2
                        t = S - 2 + s2
                        vec.wait_ge(s_act, t + 3)
                        vec.tensor_tensor(
                            ig_sb[:, :],
                            act[:, p * G : p * G + 128],
                            act[:, p * G + 384 : p * G + 512],
                            mult,
                        )
                        vec.tensor_tensor(
                            fc_sb[:, :],
                            act[:, p * G + 128 : p * G + 256],
                            c_sb[:, :],
                            mult,
                        )
                        vec.tensor_tensor(
                            c_sb[:, :], ig_sb[:, :], fc_sb[:, :], add
                        ).then_inc(s_dc)
                        vec.wait_ge(s_tc, t + 1)
                        vec.wait_ge(s_tr, t + 1)
                        vec.tensor_tensor(
                            h_sb[:, :],
                            act[:, p * G + 256 : p * G + 384],
                            tc_sb[:, p * HC : (p + 1) * HC],
                            mult,
                        ).then_inc(s_h)
                        if s2 == 0:
                            vec.wait_ge(s_tr, t + 2)
                            vec.wait_ge(s_ci[p], 16 * (S - 2) // 2)
                            vec.tensor_copy(
                                stage[:, p * B : (p + 1) * B], pt[p][:, :]
                            ).then_inc(s_ev)

            @block.gpsimd
            def _(gp):
                # straight-line collectives: rounds j = -1 .. S-2
                for j in range(-1, S - 1):
                    q = j & 1
                    # in-DMA of round j done: s_ci[q] >= 16*(m+1)
                    m = (j + 1) // 2 if q == 0 else (j + 1 + 1) // 2
                    # m: parity-q round index: q=1: j=-1,1,3..-> m=0,1,2..
                    #    q=0: j=0,2,4..  -> m=0,1,2..
                    m = (j + 1) // 2
                    gp.wait_ge(s_ci[q], 16 * (m + 1))
                    # cc_out[q] free: out-DMA of previous parity-q round done
                    if m > 0:
                        gp.wait_ge(s_hT[q], 16 * m)
                    gp.collective_compute(
                        "AllGather",
                        mybir.AluOpType.bypass,
                        replica_groups=[list(range(NC))],
                        ins=[cc_in[q * 128 : (q + 1) * 128, :].opt()],
                        outs=[
                            cc_out[q * NC * 128 : (q + 1) * NC * 128, :].opt()
                        ],
                        cc_dim="Partition",
                    ).then_inc(s_cc)

    return nc


def prep_inputs(x, W_x, W_h, b):
    assert np.allclose(b, 0.0), "kernel assumes zero biases"
    S = x.shape[1]
    Wh_r = np.transpose(np.asarray(W_h, np.float32), (1, 0, 2))
    Wx_r = np.transpose(np.asarray(W_x, np.float32), (1, 0, 2))
    xT = np.ascontiguousarray(
        np.asarray(x, np.float32).transpose(2, 1, 0).reshape(I, S * B)
    )
    ident = np.eye(B, dtype=np.float32)
    in_maps = []
    for c in range(NC):
        c0, c1 = c * HC, (c + 1) * HC
        Wh_c = np.ascontiguousarray(Wh_r[:, GATE_ORDER, c0:c1].reshape(H, G))
        Wx_c = np.ascontiguousarray(Wx_r[:, GATE_ORDER, c0:c1].reshape(I, G))
        wcat_c = np.concatenate([Wh_c, Wx_c], axis=0)
        in_maps.append({"xT": xT, "wcat": wcat_c, "ident": ident})
    return in_maps


_CACHED = {}


def kernel(x, W_x, W_h, b):
    from concourse.bass_utils import run_bass_kernel_spmd

    x = np.asarray(x, np.float32)
    in_maps = prep_inputs(
        x,
        np.asarray(W_x, np.float32),
        np.asarray(W_h, np.float32),
        np.asarray(b, np.float32),
    )
    if "nc" not in _CACHED:
        _CACHED["nc"] = build()
    res = run_bass_kernel_spmd(_CACHED["nc"], in_maps, core_ids=list(range(NC)))
    h = np.zeros((B, H), np.float32)
    c = np.zeros((B, H), np.float32)
    for ci in range(NC):
        hc = res.results[ci]["out_hc"]
        h[:, ci * HC : (ci + 1) * HC] = hc[:, :HC]
        c[:, ci * HC : (ci + 1) * HC] = hc[:, HC:]
    return h, c
